# revision 4
# baseline (speedup 1.0000x reference)
"""Cross-attention (efficient-attention variant) + 1x1 conv + LayerNorm on 8 trn2 cores.

Problem: x1,x2 [4,64,64,1024] f32. Per batch b and head h (8 heads, 128 ch each):
  value = x1[b] channel-major, kq = x2[b] channel-major
  key = softmax(kq, tokens), query = softmax(kq, head-channels)
  S = query @ key^T  [128,128];  att = S @ value  -> agg [1024, 4096]
  y = w_proj[2048,1024] @ agg + b_proj; LayerNorm(2048) * gamma + beta

Sharding: core i -> batch b=i//2, token half i%2 (2048 tokens).
Phase A (per core, all 4096 tokens of its batch): E=exp(x2 tile) once (shared by
both softmaxes); Q-hat = E * 1/rowsum per head; ST_raw[h] += E_h^T-contract-Qhat_h
accumulated over 32 token tiles in PSUM. Key-softmax normalizer: rowsum(ST_raw)
== colsum(E) because Q-hat rows sum to 1, so ST = ST_raw / rowsum(ST_raw) gives
S^T exactly.
Phase B (per core, its 2048 tokens): att_cm[h] = matmul(lhsT=ST[h], rhs=V_cm
chunk) (f32r, free=512); y = sum_h att_h^T-contract-w_projT_h (f32r, free=512);
bias-add + LayerNorm stats on free dim; gamma/beta applied on host (pure
per-channel affine after the device LN).
"""

import os
import numpy as np

import concourse.bass as bass
import concourse.tile as tile
from concourse import bacc, mybir
from concourse.bass_utils import run_bass_kernel_spmd

F32 = mybir.dt.float32
F32R = mybir.dt.float32r
BF16 = mybir.dt.bfloat16
AX = mybir.AxisListType
ALU = mybir.AluOpType
ACT_F = mybir.ActivationFunctionType

B, HI, WI, C = 4, 64, 64, 1024
N = HI * WI          # 4096 tokens per batch
HEADS = 8
CH = C // HEADS      # 128 per-head channels
C2 = 2 * C           # 2048 output channels
NCORES = 8
TOK = N // 2         # 2048 tokens per core
P = 128
NT_A = N // P        # 32 token tiles in phase A
CHUNK = 512          # phase-B token chunk (matmul free dim)
NCHUNK = TOK // CHUNK
OC = C2 // 512       # output-channel chunks of 512
EPS = 1e-5

# matmul dtype knobs (env-tunable for experiments)
MM_A = os.environ.get("K_MM_A", "f32")    # phase-A ST matmuls (free=128)
MM_B = os.environ.get("K_MM_B", "f32r")   # phase-B matmuls (free=512)

_compiled = {}


def build():
    nc = bacc.Bacc("TRN2", target_bir_lowering=False, debug=False,
                   num_devices=NCORES)
    b_dt = F32R if MM_B == "f32r" else F32
    xq = nc.dram_tensor("xq", [N, C], F32, kind="ExternalInput").ap()
    vcm = nc.dram_tensor("vcm", [C, TOK], b_dt, kind="ExternalInput").ap()
    wt = nc.dram_tensor("wt", [C, C2], b_dt, kind="ExternalInput").ap()
    brep = nc.dram_tensor("brep", [P, C2], F32, kind="ExternalInput").ap()
    y = nc.dram_tensor("y", [TOK, C2], F32, kind="ExternalOutput").ap()

    a_dt = BF16 if MM_A == "bf16" else F32

    with tile.TileContext(nc) as tc:
        with tc.tile_pool(name="persist", bufs=1) as persist:
            brep_sb = persist.tile([P, C2], F32)
            nc.sync.dma_start(brep_sb[:], brep[:])
            eps_sb = persist.tile([P, 1], F32, name="eps")
            nc.vector.memset(eps_sb[:], EPS)
            wt_sb = [persist.tile([P, C2], b_dt, name=f"wt{k}") for k in range(HEADS)]
            for k in range(HEADS):
                nc.sync.dma_start(wt_sb[k][:], wt[k * P:(k + 1) * P, :])
            sthat = [persist.tile([P, CH], b_dt, name=f"sthat{h}") for h in range(HEADS)]

            # ---------------- Phase A: S^T per head over all N tokens --------
            with tc.tile_pool(name="xq_p", bufs=3) as xq_p, \
                 tc.tile_pool(name="e_p", bufs=3) as e_p, \
                 tc.tile_pool(name="q_p", bufs=3) as q_p, \
                 tc.tile_pool(name="sm_a", bufs=4) as sm_a, \
                 tc.tile_pool(name="st_ps", bufs=1, space="PSUM") as st_psp:
                st_ps = [st_psp.tile([P, CH], F32, name=f"st{h}") for h in range(HEADS)]
                for nt in range(NT_A):
                    xt = xq_p.tile([P, C], F32)
                    nc.sync.dma_start(xt[:], xq[nt * P:(nt + 1) * P, :])
                    E = e_p.tile([P, C], a_dt)
                    nc.scalar.activation(E[:], xt[:], ACT_F.Exp)
                    qs = sm_a.tile([P, HEADS], F32)
                    nc.vector.reduce_sum(
                        qs[:], E.rearrange("p (h c) -> p h c", h=HEADS), axis=AX.X)
                    rq = sm_a.tile([P, HEADS], F32)
                    nc.vector.reciprocal(rq[:], qs[:])
                    Qh = q_p.tile([P, C], a_dt)
                    nc.vector.tensor_tensor(
                        Qh.rearrange("p (h c) -> p h c", h=HEADS),
                        E.rearrange("p (h c) -> p h c", h=HEADS),
                        rq[:, :, None].to_broadcast([P, HEADS, CH]),
                        op=ALU.mult)
                    first, last = nt == 0, nt == NT_A - 1
                    for h in range(HEADS):
                        hs = slice(h * CH, (h + 1) * CH)
                        nc.tensor.matmul(st_ps[h][:], lhsT=E[:, hs], rhs=Qh[:, hs],
                                         start=first, stop=last)
                # ST = ST_raw / rowsum(ST_raw)  (== colsum of exp-key)
                for h in range(HEADS):
                    cs = sm_a.tile([P, 1], F32, name="cs")
                    nc.vector.reduce_sum(cs[:], st_ps[h][:], axis=AX.X)
                    rc = sm_a.tile([P, 1], F32, name="rc")
                    nc.vector.reciprocal(rc[:], cs[:])
                    nc.vector.tensor_scalar_mul(sthat[h][:], st_ps[h][:], rc[:])

            # ---------------- Phase B: att, projection, LayerNorm ------------
            with tc.tile_pool(name="v_p", bufs=2) as v_p, \
                 tc.tile_pool(name="att_p", bufs=2) as att_p, \
                 tc.tile_pool(name="y_p", bufs=2) as y_p, \
                 tc.tile_pool(name="yn_p", bufs=2) as yn_p, \
                 tc.tile_pool(name="sm_b", bufs=6) as sm_b, \
                 tc.tile_pool(name="att_ps", bufs=3, space="PSUM") as att_psp, \
                 tc.tile_pool(name="y_ps", bufs=1, space="PSUM") as y_psp:
                vcm_r = vcm.rearrange("(h p) n -> p h n", p=P)
                for ck in range(NCHUNK):
                    vt = v_p.tile([P, HEADS * CHUNK], b_dt)
                    nc.sync.dma_start(
                        vt.rearrange("p (h n) -> p h n", h=HEADS),
                        vcm_r[:, :, ck * CHUNK:(ck + 1) * CHUNK])
                    att_sb = att_p.tile([P, HEADS * CHUNK], b_dt)
                    for h in range(HEADS):
                        aps = att_psp.tile([P, CHUNK], F32)
                        nc.tensor.matmul(
                            aps[:],
                            lhsT=sthat[h][:],
                            rhs=vt[:, h * CHUNK:(h + 1) * CHUNK],
                            start=True, stop=True)
                        nc.scalar.copy(att_sb[:, h * CHUNK:(h + 1) * CHUNK], aps[:])
                    for sub in range(CHUNK // P):
                        y_ps = y_psp.tile([P, C2], F32)
                        for k in range(HEADS):
                            lt = att_sb[:, k * CHUNK + sub * P:k * CHUNK + (sub + 1) * P]
                            for oc in range(OC):
                                nc.tensor.matmul(
                                    y_ps[:, oc * 512:(oc + 1) * 512],
                                    lhsT=lt,
                                    rhs=wt_sb[k][:, oc * 512:(oc + 1) * 512],
                                    start=(k == 0), stop=(k == HEADS - 1))
                        ysb = y_p.tile([P, C2], F32)
                        for pc in range(OC):
                            sl = slice(pc * 512, (pc + 1) * 512)
                            nc.vector.tensor_tensor(
                                ysb[:, sl], y_ps[:, sl], brep_sb[:, sl], op=ALU.add)
                        s = sm_b.tile([P, 1], F32, name="s")
                        nc.vector.reduce_sum(s[:], ysb[:], axis=AX.X)
                        negmu = sm_b.tile([P, 1], F32, name="negmu")
                        nc.vector.tensor_scalar_mul(negmu[:], s[:], -1.0 / C2)
                        yn = yn_p.tile([P, C2], F32)
                        ss = sm_b.tile([P, 1], F32, name="ss")
                        nc.scalar.activation(yn[:], ysb[:], ACT_F.Square,
                                             accum_out=ss[:])
                        m2 = sm_b.tile([P, 1], F32, name="m2")
                        nc.vector.tensor_tensor(m2[:], negmu[:], negmu[:], op=ALU.mult)
                        var = sm_b.tile([P, 1], F32, name="var")
                        nc.vector.tensor_scalar(var[:], ss[:], 1.0 / C2, m2[:],
                                                op0=ALU.mult, op1=ALU.subtract)
                        sig = sm_b.tile([P, 1], F32, name="sig")
                        nc.scalar.activation(sig[:], var[:], ACT_F.Sqrt,
                                             bias=eps_sb[:])
                        rsig = sm_b.tile([P, 1], F32, name="rsig")
                        nc.vector.reciprocal(rsig[:], sig[:])
                        nc.vector.tensor_scalar(yn[:], ysb[:], negmu[:], rsig[:],
                                                op0=ALU.add, op1=ALU.mult)
                        row = (ck * (CHUNK // P) + sub) * P
                        nc.sync.dma_start(y[row:row + P, :], yn[:])
    nc.compile()
    return nc


def _get_nc():
    key = (MM_A, MM_B)
    if key not in _compiled:
        _compiled[key] = build()
    return _compiled[key]


def run(inputs, trace=False):
    x1 = np.asarray(inputs["x1"], dtype=np.float32)
    x2 = np.asarray(inputs["x2"], dtype=np.float32)
    w_proj = np.asarray(inputs["w_proj"], dtype=np.float32)
    b_proj = np.asarray(inputs["b_proj"], dtype=np.float32)
    gamma = np.asarray(inputs["gamma"], dtype=np.float32)
    beta = np.asarray(inputs["beta"], dtype=np.float32)

    x1f = x1.reshape(B, N, C)
    x2f = x2.reshape(B, N, C)
    wt = np.ascontiguousarray(w_proj.T)                       # [C, 2C]
    brep = np.ascontiguousarray(np.broadcast_to(b_proj, (P, C2)))

    in_maps = []
    for core in range(NCORES):
        b, half = divmod(core, 2)
        vcm = np.ascontiguousarray(x1f[b].T[:, half * TOK:(half + 1) * TOK])
        in_maps.append({
            "xq": np.ascontiguousarray(x2f[b]),
            "vcm": vcm,
            "wt": wt,
            "brep": brep,
        })
    nc = _get_nc()
    res = run_bass_kernel_spmd(nc, in_maps, list(range(NCORES)), trace=trace)

    yout = np.empty((B, N, C2), np.float32)
    for core in range(NCORES):
        b, half = divmod(core, 2)
        yout[b, half * TOK:(half + 1) * TOK] = res.results[core]["y"]
    yout = yout * gamma + beta
    return yout.reshape(B, HI, WI, C2), res


def kernel(**inputs):
    out, _ = run(inputs, trace=False)
    return out


# revision 5
# speedup vs baseline: 1.1455x; 1.1455x over previous
"""Cross-attention (efficient-attention variant) + 1x1 conv + LayerNorm on 8 trn2 cores.

Problem: x1,x2 [4,64,64,1024] f32. Per batch b and head h (8 heads, 128 ch each):
  value = x1[b] channel-major, kq = x2[b] channel-major
  key = softmax(kq, tokens), query = softmax(kq, head-channels)
  S = query @ key^T  [128,128];  att = S @ value  -> agg [1024, 4096]
  y = w_proj[2048,1024] @ agg + b_proj; LayerNorm(2048) * gamma + beta

Sharding: core i -> batch b=i//2, token half i%2 (2048 tokens).
Phase A (per core, all 4096 tokens of its batch): E=exp(x2 tile) once (shared by
both softmaxes); Q-hat = E * 1/rowsum per head; ST_raw[h] += E_h^T-contract-Qhat_h
accumulated over 32 token tiles in PSUM. Key-softmax normalizer: rowsum(ST_raw)
== colsum(E) because Q-hat rows sum to 1, so ST = ST_raw / rowsum(ST_raw) gives
S^T exactly.
Phase B (per core, its 2048 tokens): att_cm[h] = matmul(lhsT=ST[h], rhs=V_cm
chunk) (f32r, free=512); y = sum_h att_h^T-contract-w_projT_h (f32r, free=512);
bias-add + LayerNorm stats on free dim; gamma/beta applied on host (pure
per-channel affine after the device LN).
"""

import os
import numpy as np

import concourse.bass as bass
import concourse.tile as tile
from concourse import bacc, mybir
from concourse.bass_utils import run_bass_kernel_spmd

F32 = mybir.dt.float32
F32R = mybir.dt.float32r
BF16 = mybir.dt.bfloat16
AX = mybir.AxisListType
ALU = mybir.AluOpType
ACT_F = mybir.ActivationFunctionType

B, HI, WI, C = 4, 64, 64, 1024
N = HI * WI          # 4096 tokens per batch
HEADS = 8
CH = C // HEADS      # 128 per-head channels
C2 = 2 * C           # 2048 output channels
NCORES = 8
TOK = N // 2         # 2048 tokens per core
P = 128
NT_A = N // P        # 32 token tiles in phase A
CHUNK = 512          # phase-B token chunk (matmul free dim)
NCHUNK = TOK // CHUNK
OC = C2 // 512       # output-channel chunks of 512
EPS = 1e-5

# matmul dtype knobs (env-tunable for experiments)
MM_A = os.environ.get("K_MM_A", "bf16")    # phase-A ST matmuls (free=128)
MM_B = os.environ.get("K_MM_B", "bf16")   # phase-B matmuls (free=512)

_compiled = {}


def build():
    nc = bacc.Bacc("TRN2", target_bir_lowering=False, debug=False,
                   num_devices=NCORES)
    b_dt = {"f32r": F32R, "bf16": BF16}.get(MM_B, F32)
    xq = nc.dram_tensor("xq", [N, C], F32, kind="ExternalInput").ap()
    vcm = nc.dram_tensor("vcm", [C, TOK], b_dt, kind="ExternalInput").ap()
    wt = nc.dram_tensor("wt", [C, C2], b_dt, kind="ExternalInput").ap()
    brep = nc.dram_tensor("brep", [P, C2], F32, kind="ExternalInput").ap()
    y = nc.dram_tensor("y", [TOK, C2], F32, kind="ExternalOutput").ap()

    a_dt = BF16 if MM_A == "bf16" else F32

    with tile.TileContext(nc) as tc:
        with tc.tile_pool(name="persist", bufs=1) as persist:
            brep_sb = persist.tile([P, C2], F32)
            nc.sync.dma_start(brep_sb[:], brep[:])
            eps_sb = persist.tile([P, 1], F32, name="eps")
            nc.vector.memset(eps_sb[:], EPS)
            wt_sb = [persist.tile([P, C2], b_dt, name=f"wt{k}") for k in range(HEADS)]
            for k in range(HEADS):
                nc.sync.dma_start(wt_sb[k][:], wt[k * P:(k + 1) * P, :])
            sthat = [persist.tile([P, CH], b_dt, name=f"sthat{h}") for h in range(HEADS)]

            # ---------------- Phase A: S^T per head over all N tokens --------
            with tc.tile_pool(name="xq_p", bufs=3) as xq_p, \
                 tc.tile_pool(name="e_p", bufs=3) as e_p, \
                 tc.tile_pool(name="q_p", bufs=3) as q_p, \
                 tc.tile_pool(name="sm_a", bufs=4) as sm_a, \
                 tc.tile_pool(name="st_ps", bufs=1, space="PSUM") as st_psp:
                st_ps = [st_psp.tile([P, CH], F32, name=f"st{h}") for h in range(HEADS)]
                for nt in range(NT_A):
                    xt = xq_p.tile([P, C], F32)
                    nc.sync.dma_start(xt[:], xq[nt * P:(nt + 1) * P, :])
                    E = e_p.tile([P, C], a_dt)
                    nc.scalar.activation(E[:], xt[:], ACT_F.Exp)
                    qs = sm_a.tile([P, HEADS], F32)
                    nc.vector.reduce_sum(
                        qs[:], E.rearrange("p (h c) -> p h c", h=HEADS), axis=AX.X)
                    rq = sm_a.tile([P, HEADS], F32)
                    nc.vector.reciprocal(rq[:], qs[:])
                    Qh = q_p.tile([P, C], a_dt)
                    nc.vector.tensor_tensor(
                        Qh.rearrange("p (h c) -> p h c", h=HEADS),
                        E.rearrange("p (h c) -> p h c", h=HEADS),
                        rq[:, :, None].to_broadcast([P, HEADS, CH]),
                        op=ALU.mult)
                    first, last = nt == 0, nt == NT_A - 1
                    for h in range(HEADS):
                        hs = slice(h * CH, (h + 1) * CH)
                        nc.tensor.matmul(st_ps[h][:], lhsT=E[:, hs], rhs=Qh[:, hs],
                                         start=first, stop=last)
                # ST = ST_raw / rowsum(ST_raw)  (== colsum of exp-key)
                for h in range(HEADS):
                    cs = sm_a.tile([P, 1], F32, name="cs")
                    nc.vector.reduce_sum(cs[:], st_ps[h][:], axis=AX.X)
                    rc = sm_a.tile([P, 1], F32, name="rc")
                    nc.vector.reciprocal(rc[:], cs[:])
                    nc.vector.tensor_scalar_mul(sthat[h][:], st_ps[h][:], rc[:])

            # ---------------- Phase B: att, projection, LayerNorm ------------
            with tc.tile_pool(name="v_p", bufs=2) as v_p, \
                 tc.tile_pool(name="att_p", bufs=2) as att_p, \
                 tc.tile_pool(name="y_p", bufs=2) as y_p, \
                 tc.tile_pool(name="yn_p", bufs=2) as yn_p, \
                 tc.tile_pool(name="sm_b", bufs=6) as sm_b, \
                 tc.tile_pool(name="att_ps", bufs=3, space="PSUM") as att_psp, \
                 tc.tile_pool(name="y_ps", bufs=1, space="PSUM") as y_psp:
                vcm_r = vcm.rearrange("(h p) n -> p h n", p=P)
                for ck in range(NCHUNK):
                    vt = v_p.tile([P, HEADS * CHUNK], b_dt)
                    nc.sync.dma_start(
                        vt.rearrange("p (h n) -> p h n", h=HEADS),
                        vcm_r[:, :, ck * CHUNK:(ck + 1) * CHUNK])
                    att_sb = att_p.tile([P, HEADS * CHUNK], b_dt)
                    for h in range(HEADS):
                        aps = att_psp.tile([P, CHUNK], F32)
                        nc.tensor.matmul(
                            aps[:],
                            lhsT=sthat[h][:],
                            rhs=vt[:, h * CHUNK:(h + 1) * CHUNK],
                            start=True, stop=True)
                        nc.scalar.copy(att_sb[:, h * CHUNK:(h + 1) * CHUNK], aps[:])
                    for sub in range(CHUNK // P):
                        y_ps = y_psp.tile([P, C2], F32)
                        for k in range(HEADS):
                            lt = att_sb[:, k * CHUNK + sub * P:k * CHUNK + (sub + 1) * P]
                            for oc in range(OC):
                                nc.tensor.matmul(
                                    y_ps[:, oc * 512:(oc + 1) * 512],
                                    lhsT=lt,
                                    rhs=wt_sb[k][:, oc * 512:(oc + 1) * 512],
                                    start=(k == 0), stop=(k == HEADS - 1))
                        ysb = y_p.tile([P, C2], F32)
                        for pc in range(OC):
                            sl = slice(pc * 512, (pc + 1) * 512)
                            nc.vector.tensor_tensor(
                                ysb[:, sl], y_ps[:, sl], brep_sb[:, sl], op=ALU.add)
                        s = sm_b.tile([P, 1], F32, name="s")
                        nc.vector.reduce_sum(s[:], ysb[:], axis=AX.X)
                        negmu = sm_b.tile([P, 1], F32, name="negmu")
                        nc.vector.tensor_scalar_mul(negmu[:], s[:], -1.0 / C2)
                        yn = yn_p.tile([P, C2], F32)
                        ss = sm_b.tile([P, 1], F32, name="ss")
                        nc.scalar.activation(yn[:], ysb[:], ACT_F.Square,
                                             accum_out=ss[:])
                        m2 = sm_b.tile([P, 1], F32, name="m2")
                        nc.vector.tensor_tensor(m2[:], negmu[:], negmu[:], op=ALU.mult)
                        var = sm_b.tile([P, 1], F32, name="var")
                        nc.vector.tensor_scalar(var[:], ss[:], 1.0 / C2, m2[:],
                                                op0=ALU.mult, op1=ALU.subtract)
                        sig = sm_b.tile([P, 1], F32, name="sig")
                        nc.scalar.activation(sig[:], var[:], ACT_F.Sqrt,
                                             bias=eps_sb[:])
                        rsig = sm_b.tile([P, 1], F32, name="rsig")
                        nc.vector.reciprocal(rsig[:], sig[:])
                        nc.vector.tensor_scalar(yn[:], ysb[:], negmu[:], rsig[:],
                                                op0=ALU.add, op1=ALU.mult)
                        row = (ck * (CHUNK // P) + sub) * P
                        nc.sync.dma_start(y[row:row + P, :], yn[:])
    nc.compile()
    return nc


def _get_nc():
    key = (MM_A, MM_B)
    if key not in _compiled:
        _compiled[key] = build()
    return _compiled[key]


def run(inputs, trace=False):
    x1 = np.asarray(inputs["x1"], dtype=np.float32)
    x2 = np.asarray(inputs["x2"], dtype=np.float32)
    w_proj = np.asarray(inputs["w_proj"], dtype=np.float32)
    b_proj = np.asarray(inputs["b_proj"], dtype=np.float32)
    gamma = np.asarray(inputs["gamma"], dtype=np.float32)
    beta = np.asarray(inputs["beta"], dtype=np.float32)

    x1f = x1.reshape(B, N, C)
    x2f = x2.reshape(B, N, C)
    wt = np.ascontiguousarray(w_proj.T)                       # [C, 2C]
    brep = np.ascontiguousarray(np.broadcast_to(b_proj, (P, C2)))
    if MM_B == "bf16":
        import ml_dtypes
        wt = wt.astype(ml_dtypes.bfloat16)

    in_maps = []
    for core in range(NCORES):
        b, half = divmod(core, 2)
        vcm = np.ascontiguousarray(x1f[b].T[:, half * TOK:(half + 1) * TOK])
        if MM_B == "bf16":
            import ml_dtypes
            vcm = vcm.astype(ml_dtypes.bfloat16)
        in_maps.append({
            "xq": np.ascontiguousarray(x2f[b]),
            "vcm": vcm,
            "wt": wt,
            "brep": brep,
        })
    nc = _get_nc()
    res = run_bass_kernel_spmd(nc, in_maps, list(range(NCORES)), trace=trace)

    yout = np.empty((B, N, C2), np.float32)
    for core in range(NCORES):
        b, half = divmod(core, 2)
        yout[b, half * TOK:(half + 1) * TOK] = res.results[core]["y"]
    yout = yout * gamma + beta
    return yout.reshape(B, HI, WI, C2), res


def kernel(**inputs):
    out, _ = run(inputs, trace=False)
    return out


# revision 9
# speedup vs baseline: 1.3714x; 1.1973x over previous
"""Cross-attention (efficient-attention variant) + 1x1 conv + LayerNorm on 8 trn2 cores.

Problem: x1,x2 [4,64,64,1024] f32. Per batch b and head h (8 heads, 128 ch each):
  value = x1[b] channel-major, kq = x2[b] channel-major
  key = softmax(kq, tokens), query = softmax(kq, head-channels)
  S = query @ key^T  [128,128];  att = S @ value  -> agg [1024, 4096]
  y = w_proj[2048,1024] @ agg + b_proj; LayerNorm(2048) * gamma + beta

Sharding: core i -> batch b=i//2, token half i%2 (2048 tokens).
Phase A (per core, all 4096 tokens of its batch): E=exp(x2 tile) once (shared by
both softmaxes); Q-hat = E * 1/rowsum per head; ST_raw[h] += E_h^T-contract-Qhat_h
accumulated over 32 token tiles in PSUM. Key-softmax normalizer: rowsum(ST_raw)
== colsum(E) because Q-hat rows sum to 1, so ST = ST_raw / rowsum(ST_raw) gives
S^T exactly.
Phase B (per core, its 2048 tokens): att_cm[h] = matmul(lhsT=ST[h], rhs=V_cm
chunk) (f32r, free=512); y = sum_h att_h^T-contract-w_projT_h (f32r, free=512);
bias-add + LayerNorm stats on free dim; gamma/beta applied on host (pure
per-channel affine after the device LN).
"""

import os
import numpy as np

import concourse.bass as bass
import concourse.tile as tile
from concourse import bacc, mybir
from concourse.bass_utils import run_bass_kernel_spmd

F32 = mybir.dt.float32
F32R = mybir.dt.float32r
BF16 = mybir.dt.bfloat16
AX = mybir.AxisListType
ALU = mybir.AluOpType
ACT_F = mybir.ActivationFunctionType

B, HI, WI, C = 4, 64, 64, 1024
N = HI * WI          # 4096 tokens per batch
HEADS = 8
CH = C // HEADS      # 128 per-head channels
C2 = 2 * C           # 2048 output channels
NCORES = 8
TOK = N // 2         # 2048 tokens per core
P = 128
NT_A = N // P        # 32 token tiles in phase A
CHUNK = 512          # phase-B token chunk (matmul free dim)
NCHUNK = TOK // CHUNK
OC = C2 // 512       # output-channel chunks of 512
EPS = 1e-5

# matmul dtype knobs (env-tunable for experiments)
MM_A = os.environ.get("K_MM_A", "bf16")    # phase-A ST matmuls (free=128)
MM_B = os.environ.get("K_MM_B", "bf16")   # phase-B matmuls (free=512)
USE_TTR = os.environ.get("K_TTR", "1") == "1"

_compiled = {}


def build():
    nc = bacc.Bacc("TRN2", target_bir_lowering=False, debug=False,
                   num_devices=NCORES)
    b_dt = {"f32r": F32R, "bf16": BF16}.get(MM_B, F32)
    xq = nc.dram_tensor("xq", [N, C], F32, kind="ExternalInput").ap()
    vcm = nc.dram_tensor("vcm", [C, TOK], b_dt, kind="ExternalInput").ap()
    wt = nc.dram_tensor("wt", [C, C2], b_dt, kind="ExternalInput").ap()
    brep = nc.dram_tensor("brep", [P, C2], F32, kind="ExternalInput").ap()
    y = nc.dram_tensor("y", [TOK, C2], F32, kind="ExternalOutput").ap()

    a_dt = BF16 if MM_A == "bf16" else F32

    with tile.TileContext(nc) as tc:
        with tc.tile_pool(name="persist", bufs=1) as persist:
            brep_sb = persist.tile([P, C2], F32)
            nc.sync.dma_start(brep_sb[:], brep[:])
            eps_sb = persist.tile([P, 1], F32, name="eps")
            nc.vector.memset(eps_sb[:], EPS)
            wt_sb = [persist.tile([P, C2], b_dt, name=f"wt{k}") for k in range(HEADS)]
            for k in range(HEADS):
                nc.sync.dma_start(wt_sb[k][:], wt[k * P:(k + 1) * P, :])
            sthat = [persist.tile([P, CH], b_dt, name=f"sthat{h}") for h in range(HEADS)]

            # ---------------- Phase A: S^T per head over all N tokens --------
            with tc.tile_pool(name="xq_p", bufs=3) as xq_p, \
                 tc.tile_pool(name="e_p", bufs=3) as e_p, \
                 tc.tile_pool(name="q_p", bufs=3) as q_p, \
                 tc.tile_pool(name="sm_a", bufs=4) as sm_a, \
                 tc.tile_pool(name="st_ps", bufs=1, space="PSUM") as st_psp:
                st_ps = [st_psp.tile([P, CH], F32, name=f"st{h}") for h in range(HEADS)]
                for nt in range(NT_A):
                    xt = xq_p.tile([P, C], F32)
                    nc.sync.dma_start(xt[:], xq[nt * P:(nt + 1) * P, :])
                    E = e_p.tile([P, C], a_dt)
                    nc.scalar.activation(E[:], xt[:], ACT_F.Exp)
                    qs = sm_a.tile([P, HEADS], F32)
                    nc.vector.reduce_sum(
                        qs[:], E.rearrange("p (h c) -> p h c", h=HEADS), axis=AX.X)
                    rq = sm_a.tile([P, HEADS], F32)
                    nc.vector.reciprocal(rq[:], qs[:])
                    Qh = q_p.tile([P, C], a_dt)
                    GH = int(os.environ.get('K_GH', '5'))  # heads on GpSimd
                    DH = HEADS - GH
                    nc.vector.tensor_tensor(
                        Qh.rearrange("p (h c) -> p h c", h=HEADS)[:, :DH],
                        E.rearrange("p (h c) -> p h c", h=HEADS)[:, :DH],
                        rq[:, :DH, None].to_broadcast([P, DH, CH]),
                        op=ALU.mult)
                    if GH:
                        nc.gpsimd.tensor_tensor(
                            Qh.rearrange("p (h c) -> p h c", h=HEADS)[:, DH:],
                            E.rearrange("p (h c) -> p h c", h=HEADS)[:, DH:],
                            rq[:, DH:, None].to_broadcast([P, GH, CH]),
                            op=ALU.mult)
                    first, last = nt == 0, nt == NT_A - 1
                    for h in range(HEADS):
                        hs = slice(h * CH, (h + 1) * CH)
                        nc.tensor.matmul(st_ps[h][:], lhsT=E[:, hs], rhs=Qh[:, hs],
                                         start=first, stop=last)
                # ST = ST_raw / rowsum(ST_raw)  (== colsum of exp-key)
                for h in range(HEADS):
                    cs = sm_a.tile([P, 1], F32, name="cs")
                    nc.vector.reduce_sum(cs[:], st_ps[h][:], axis=AX.X)
                    rc = sm_a.tile([P, 1], F32, name="rc")
                    nc.vector.reciprocal(rc[:], cs[:])
                    nc.vector.tensor_scalar_mul(sthat[h][:], st_ps[h][:], rc[:])

            # ---------------- Phase B: att, projection, LayerNorm ------------
            with tc.tile_pool(name="v_p", bufs=2) as v_p, \
                 tc.tile_pool(name="att_p", bufs=2) as att_p, \
                 tc.tile_pool(name="y_p", bufs=2) as y_p, \
                 tc.tile_pool(name="yn_p", bufs=2) as yn_p, \
                 tc.tile_pool(name="sm_b", bufs=6) as sm_b, \
                 tc.tile_pool(name="ps_b", bufs=8, space="PSUM") as ps_b:
                vcm_r = vcm.rearrange("(h p) n -> p h n", p=P)
                for ck in range(NCHUNK):
                    vt = v_p.tile([P, HEADS * CHUNK], b_dt)
                    nc.sync.dma_start(
                        vt.rearrange("p (h n) -> p h n", h=HEADS),
                        vcm_r[:, :, ck * CHUNK:(ck + 1) * CHUNK])
                    att_sb = att_p.tile([P, HEADS * CHUNK], b_dt)
                    for h in range(HEADS):
                        aps = ps_b.tile([P, CHUNK], F32, tag="ps", name=f"aps{ck}_{h}")
                        nc.tensor.matmul(
                            aps[:],
                            lhsT=sthat[h][:],
                            rhs=vt[:, h * CHUNK:(h + 1) * CHUNK],
                            start=True, stop=True)
                        nc.scalar.copy(att_sb[:, h * CHUNK:(h + 1) * CHUNK], aps[:])
                    for sub in range(CHUNK // P):
                        yps = [ps_b.tile([P, 512], F32, tag="ps", name=f"yps{ck}_{sub}_{o}") for o in range(OC)]
                        for k in range(HEADS):
                            lt = att_sb[:, k * CHUNK + sub * P:k * CHUNK + (sub + 1) * P]
                            for oc in range(OC):
                                nc.tensor.matmul(
                                    yps[oc][:],
                                    lhsT=lt,
                                    rhs=wt_sb[k][:, oc * 512:(oc + 1) * 512],
                                    start=(k == 0), stop=(k == HEADS - 1))
                        ysb = y_p.tile([P, C2], F32)
                        negmu = sm_b.tile([P, 1], F32, name="negmu")
                        if USE_TTR:
                            spart = sm_b.tile([P, OC], F32, name="spart")
                            for pc in range(OC):
                                sl = slice(pc * 512, (pc + 1) * 512)
                                nc.vector.tensor_tensor_reduce(
                                    ysb[:, sl], yps[pc][:], brep_sb[:, sl],
                                    scale=1.0, scalar=0.0,
                                    op0=ALU.add, op1=ALU.add,
                                    accum_out=spart[:, pc:pc + 1])
                            nc.vector.tensor_reduce(spart[:, 0:1], spart[:], axis=AX.X,
                                                    op=ALU.add)
                            nc.vector.tensor_scalar_mul(negmu[:], spart[:, 0:1],
                                                        -1.0 / C2)
                        else:
                            for pc in range(OC):
                                sl = slice(pc * 512, (pc + 1) * 512)
                                nc.vector.tensor_tensor(
                                    ysb[:, sl], yps[pc][:], brep_sb[:, sl],
                                    op=ALU.add)
                            s_ = sm_b.tile([P, 1], F32, name="s_")
                            nc.vector.reduce_sum(s_[:], ysb[:], axis=AX.X)
                            nc.vector.tensor_scalar_mul(negmu[:], s_[:], -1.0 / C2)
                        yn = yn_p.tile([P, C2], F32)
                        ss = sm_b.tile([P, 1], F32, name="ss")
                        nc.scalar.activation(yn[:], ysb[:], ACT_F.Square,
                                             accum_out=ss[:])
                        m2 = sm_b.tile([P, 1], F32, name="m2")
                        nc.vector.tensor_tensor(m2[:], negmu[:], negmu[:], op=ALU.mult)
                        var = sm_b.tile([P, 1], F32, name="var")
                        nc.vector.tensor_scalar(var[:], ss[:], 1.0 / C2, m2[:],
                                                op0=ALU.mult, op1=ALU.subtract)
                        sig = sm_b.tile([P, 1], F32, name="sig")
                        nc.scalar.activation(sig[:], var[:], ACT_F.Sqrt,
                                             bias=eps_sb[:])
                        rsig = sm_b.tile([P, 1], F32, name="rsig")
                        nc.vector.reciprocal(rsig[:], sig[:])
                        nc.vector.tensor_scalar(yn[:], ysb[:], negmu[:], rsig[:],
                                                op0=ALU.add, op1=ALU.mult)
                        row = (ck * (CHUNK // P) + sub) * P
                        nc.sync.dma_start(y[row:row + P, :], yn[:])
    nc.compile()
    return nc


def _get_nc():
    key = (MM_A, MM_B, USE_TTR)
    if key not in _compiled:
        _compiled[key] = build()
    return _compiled[key]


def run(inputs, trace=False):
    x1 = np.asarray(inputs["x1"], dtype=np.float32)
    x2 = np.asarray(inputs["x2"], dtype=np.float32)
    w_proj = np.asarray(inputs["w_proj"], dtype=np.float32)
    b_proj = np.asarray(inputs["b_proj"], dtype=np.float32)
    gamma = np.asarray(inputs["gamma"], dtype=np.float32)
    beta = np.asarray(inputs["beta"], dtype=np.float32)

    x1f = x1.reshape(B, N, C)
    x2f = x2.reshape(B, N, C)
    wt = np.ascontiguousarray(w_proj.T)                       # [C, 2C]
    brep = np.ascontiguousarray(np.broadcast_to(b_proj, (P, C2)))
    if MM_B == "bf16":
        import ml_dtypes
        wt = wt.astype(ml_dtypes.bfloat16)

    in_maps = []
    for core in range(NCORES):
        b, half = divmod(core, 2)
        vcm = np.ascontiguousarray(x1f[b].T[:, half * TOK:(half + 1) * TOK])
        if MM_B == "bf16":
            import ml_dtypes
            vcm = vcm.astype(ml_dtypes.bfloat16)
        in_maps.append({
            "xq": np.ascontiguousarray(x2f[b]),
            "vcm": vcm,
            "wt": wt,
            "brep": brep,
        })
    nc = _get_nc()
    res = run_bass_kernel_spmd(nc, in_maps, list(range(NCORES)), trace=trace)

    yout = np.empty((B, N, C2), np.float32)
    for core in range(NCORES):
        b, half = divmod(core, 2)
        yout[b, half * TOK:(half + 1) * TOK] = res.results[core]["y"]
    yout = yout * gamma + beta
    return yout.reshape(B, HI, WI, C2), res


def kernel(**inputs):
    out, _ = run(inputs, trace=False)
    return out


# revision 10
# speedup vs baseline: 1.3749x; 1.0025x over previous
"""Cross-attention (efficient-attention variant) + 1x1 conv + LayerNorm on 8 trn2 cores.

Problem: x1,x2 [4,64,64,1024] f32. Per batch b and head h (8 heads, 128 ch each):
  value = x1[b] channel-major, kq = x2[b] channel-major
  key = softmax(kq, tokens), query = softmax(kq, head-channels)
  S = query @ key^T  [128,128];  att = S @ value  -> agg [1024, 4096]
  y = w_proj[2048,1024] @ agg + b_proj; LayerNorm(2048) * gamma + beta

Sharding: core i -> batch b=i//2, token half i%2 (2048 tokens).
Phase A (per core, all 4096 tokens of its batch): E=exp(x2 tile) once (shared by
both softmaxes); Q-hat = E * 1/rowsum per head; ST_raw[h] += E_h^T-contract-Qhat_h
accumulated over 32 token tiles in PSUM. Key-softmax normalizer: rowsum(ST_raw)
== colsum(E) because Q-hat rows sum to 1, so ST = ST_raw / rowsum(ST_raw) gives
S^T exactly.
Phase B (per core, its 2048 tokens): att_cm[h] = matmul(lhsT=ST[h], rhs=V_cm
chunk) (f32r, free=512); y = sum_h att_h^T-contract-w_projT_h (f32r, free=512);
bias-add + LayerNorm stats on free dim; gamma/beta applied on host (pure
per-channel affine after the device LN).
"""

import os
import numpy as np

import concourse.bass as bass
import concourse.tile as tile
from concourse import bacc, mybir
from concourse.bass_utils import run_bass_kernel_spmd

F32 = mybir.dt.float32
F32R = mybir.dt.float32r
BF16 = mybir.dt.bfloat16
AX = mybir.AxisListType
ALU = mybir.AluOpType
ACT_F = mybir.ActivationFunctionType

B, HI, WI, C = 4, 64, 64, 1024
N = HI * WI          # 4096 tokens per batch
HEADS = 8
CH = C // HEADS      # 128 per-head channels
C2 = 2 * C           # 2048 output channels
NCORES = 8
TOK = N // 2         # 2048 tokens per core
P = 128
NT_A = N // P        # 32 token tiles in phase A
CHUNK = 512          # phase-B token chunk (matmul free dim)
NCHUNK = TOK // CHUNK
OC = C2 // 512       # output-channel chunks of 512
EPS = 1e-5

# matmul dtype knobs (env-tunable for experiments)
MM_A = os.environ.get("K_MM_A", "bf16")    # phase-A ST matmuls (free=128)
MM_B = os.environ.get("K_MM_B", "bf16")   # phase-B matmuls (free=512)
USE_TTR = os.environ.get("K_TTR", "0") == "1"

_compiled = {}


def build():
    nc = bacc.Bacc("TRN2", target_bir_lowering=False, debug=False,
                   num_devices=NCORES)
    b_dt = {"f32r": F32R, "bf16": BF16}.get(MM_B, F32)
    xq = nc.dram_tensor("xq", [N, C], F32, kind="ExternalInput").ap()
    vcm = nc.dram_tensor("vcm", [C, TOK], b_dt, kind="ExternalInput").ap()
    wt = nc.dram_tensor("wt", [C, C2], b_dt, kind="ExternalInput").ap()
    brep = nc.dram_tensor("brep", [P, C2], F32, kind="ExternalInput").ap()
    y = nc.dram_tensor("y", [TOK, C2], F32, kind="ExternalOutput").ap()

    a_dt = BF16 if MM_A == "bf16" else F32

    with tile.TileContext(nc) as tc:
        with tc.tile_pool(name="persist", bufs=1) as persist:
            brep_sb = persist.tile([P, C2], F32)
            nc.sync.dma_start(brep_sb[:], brep[:])
            eps_sb = persist.tile([P, 1], F32, name="eps")
            nc.vector.memset(eps_sb[:], EPS)
            wt_sb = [persist.tile([P, C2], b_dt, name=f"wt{k}") for k in range(HEADS)]
            for k in range(HEADS):
                nc.sync.dma_start(wt_sb[k][:], wt[k * P:(k + 1) * P, :])
            sthat = [persist.tile([P, CH], b_dt, name=f"sthat{h}") for h in range(HEADS)]

            # ---------------- Phase A: S^T per head over all N tokens --------
            with tc.tile_pool(name="xq_p", bufs=3) as xq_p, \
                 tc.tile_pool(name="e_p", bufs=3) as e_p, \
                 tc.tile_pool(name="q_p", bufs=3) as q_p, \
                 tc.tile_pool(name="sm_a", bufs=4) as sm_a, \
                 tc.tile_pool(name="st_ps", bufs=1, space="PSUM") as st_psp:
                st_ps = [st_psp.tile([P, CH], F32, name=f"st{h}") for h in range(HEADS)]
                for nt in range(NT_A):
                    xt = xq_p.tile([P, C], F32)
                    nc.sync.dma_start(xt[:], xq[nt * P:(nt + 1) * P, :])
                    E = e_p.tile([P, C], a_dt)
                    nc.scalar.activation(E[:], xt[:], ACT_F.Exp)
                    qs = sm_a.tile([P, HEADS], F32)
                    nc.vector.reduce_sum(
                        qs[:], E.rearrange("p (h c) -> p h c", h=HEADS), axis=AX.X)
                    rq = sm_a.tile([P, HEADS], F32)
                    nc.vector.reciprocal(rq[:], qs[:])
                    Qh = q_p.tile([P, C], a_dt)
                    GH = int(os.environ.get('K_GH', '5'))  # heads on GpSimd
                    DH = HEADS - GH
                    nc.vector.tensor_tensor(
                        Qh.rearrange("p (h c) -> p h c", h=HEADS)[:, :DH],
                        E.rearrange("p (h c) -> p h c", h=HEADS)[:, :DH],
                        rq[:, :DH, None].to_broadcast([P, DH, CH]),
                        op=ALU.mult)
                    if GH:
                        nc.gpsimd.tensor_tensor(
                            Qh.rearrange("p (h c) -> p h c", h=HEADS)[:, DH:],
                            E.rearrange("p (h c) -> p h c", h=HEADS)[:, DH:],
                            rq[:, DH:, None].to_broadcast([P, GH, CH]),
                            op=ALU.mult)
                    first, last = nt == 0, nt == NT_A - 1
                    for h in range(HEADS):
                        hs = slice(h * CH, (h + 1) * CH)
                        nc.tensor.matmul(st_ps[h][:], lhsT=E[:, hs], rhs=Qh[:, hs],
                                         start=first, stop=last)
                # ST = ST_raw / rowsum(ST_raw)  (== colsum of exp-key)
                for h in range(HEADS):
                    cs = sm_a.tile([P, 1], F32, name="cs")
                    nc.vector.reduce_sum(cs[:], st_ps[h][:], axis=AX.X)
                    rc = sm_a.tile([P, 1], F32, name="rc")
                    nc.vector.reciprocal(rc[:], cs[:])
                    nc.vector.tensor_scalar_mul(sthat[h][:], st_ps[h][:], rc[:])

            # ---------------- Phase B: att, projection, LayerNorm ------------
            with tc.tile_pool(name="v_p", bufs=2) as v_p, \
                 tc.tile_pool(name="att_p", bufs=2) as att_p, \
                 tc.tile_pool(name="y_p", bufs=2) as y_p, \
                 tc.tile_pool(name="yn_p", bufs=2) as yn_p, \
                 tc.tile_pool(name="sm_b", bufs=6) as sm_b, \
                 tc.tile_pool(name="ps_b", bufs=8, space="PSUM") as ps_b:
                vcm_r = vcm.rearrange("(h p) n -> p h n", p=P)
                for ck in range(NCHUNK):
                    vt = v_p.tile([P, HEADS * CHUNK], b_dt)
                    nc.sync.dma_start(
                        vt.rearrange("p (h n) -> p h n", h=HEADS),
                        vcm_r[:, :, ck * CHUNK:(ck + 1) * CHUNK])
                    att_sb = att_p.tile([P, HEADS * CHUNK], b_dt)
                    for h in range(HEADS):
                        aps = ps_b.tile([P, CHUNK], F32, tag="ps", name=f"aps{ck}_{h}")
                        nc.tensor.matmul(
                            aps[:],
                            lhsT=sthat[h][:],
                            rhs=vt[:, h * CHUNK:(h + 1) * CHUNK],
                            start=True, stop=True)
                        nc.scalar.copy(att_sb[:, h * CHUNK:(h + 1) * CHUNK], aps[:])
                    for sub in range(CHUNK // P):
                        yps = [ps_b.tile([P, 512], F32, tag="ps", name=f"yps{ck}_{sub}_{o}") for o in range(OC)]
                        for k in range(HEADS):
                            lt = att_sb[:, k * CHUNK + sub * P:k * CHUNK + (sub + 1) * P]
                            for oc in range(OC):
                                nc.tensor.matmul(
                                    yps[oc][:],
                                    lhsT=lt,
                                    rhs=wt_sb[k][:, oc * 512:(oc + 1) * 512],
                                    start=(k == 0), stop=(k == HEADS - 1))
                        ysb = y_p.tile([P, C2], F32)
                        negmu = sm_b.tile([P, 1], F32, name="negmu")
                        if USE_TTR:
                            spart = sm_b.tile([P, OC], F32, name="spart")
                            for pc in range(OC):
                                sl = slice(pc * 512, (pc + 1) * 512)
                                nc.vector.tensor_tensor_reduce(
                                    ysb[:, sl], yps[pc][:], brep_sb[:, sl],
                                    scale=1.0, scalar=0.0,
                                    op0=ALU.add, op1=ALU.add,
                                    accum_out=spart[:, pc:pc + 1])
                            nc.vector.tensor_reduce(spart[:, 0:1], spart[:], axis=AX.X,
                                                    op=ALU.add)
                            nc.vector.tensor_scalar_mul(negmu[:], spart[:, 0:1],
                                                        -1.0 / C2)
                        else:
                            for pc in range(OC):
                                sl = slice(pc * 512, (pc + 1) * 512)
                                nc.vector.tensor_tensor(
                                    ysb[:, sl], yps[pc][:], brep_sb[:, sl],
                                    op=ALU.add)
                            s_ = sm_b.tile([P, 1], F32, name="s_")
                            nc.vector.reduce_sum(s_[:], ysb[:], axis=AX.X)
                            nc.vector.tensor_scalar_mul(negmu[:], s_[:], -1.0 / C2)
                        yn = yn_p.tile([P, C2], F32)
                        ss = sm_b.tile([P, 1], F32, name="ss")
                        nc.scalar.activation(yn[:], ysb[:], ACT_F.Square,
                                             accum_out=ss[:])
                        m2 = sm_b.tile([P, 1], F32, name="m2")
                        nc.vector.tensor_tensor(m2[:], negmu[:], negmu[:], op=ALU.mult)
                        var = sm_b.tile([P, 1], F32, name="var")
                        nc.vector.tensor_scalar(var[:], ss[:], 1.0 / C2, m2[:],
                                                op0=ALU.mult, op1=ALU.subtract)
                        sig = sm_b.tile([P, 1], F32, name="sig")
                        nc.scalar.activation(sig[:], var[:], ACT_F.Sqrt,
                                             bias=eps_sb[:])
                        rsig = sm_b.tile([P, 1], F32, name="rsig")
                        nc.vector.reciprocal(rsig[:], sig[:])
                        nc.vector.tensor_scalar(yn[:], ysb[:], negmu[:], rsig[:],
                                                op0=ALU.add, op1=ALU.mult)
                        row = (ck * (CHUNK // P) + sub) * P
                        nc.sync.dma_start(y[row:row + P, :], yn[:])
    nc.compile()
    return nc


def _get_nc():
    key = (MM_A, MM_B, USE_TTR)
    if key not in _compiled:
        _compiled[key] = build()
    return _compiled[key]


def run(inputs, trace=False):
    x1 = np.asarray(inputs["x1"], dtype=np.float32)
    x2 = np.asarray(inputs["x2"], dtype=np.float32)
    w_proj = np.asarray(inputs["w_proj"], dtype=np.float32)
    b_proj = np.asarray(inputs["b_proj"], dtype=np.float32)
    gamma = np.asarray(inputs["gamma"], dtype=np.float32)
    beta = np.asarray(inputs["beta"], dtype=np.float32)

    x1f = x1.reshape(B, N, C)
    x2f = x2.reshape(B, N, C)
    wt = np.ascontiguousarray(w_proj.T)                       # [C, 2C]
    brep = np.ascontiguousarray(np.broadcast_to(b_proj, (P, C2)))
    if MM_B == "bf16":
        import ml_dtypes
        wt = wt.astype(ml_dtypes.bfloat16)

    in_maps = []
    for core in range(NCORES):
        b, half = divmod(core, 2)
        vcm = np.ascontiguousarray(x1f[b].T[:, half * TOK:(half + 1) * TOK])
        if MM_B == "bf16":
            import ml_dtypes
            vcm = vcm.astype(ml_dtypes.bfloat16)
        in_maps.append({
            "xq": np.ascontiguousarray(x2f[b]),
            "vcm": vcm,
            "wt": wt,
            "brep": brep,
        })
    nc = _get_nc()
    res = run_bass_kernel_spmd(nc, in_maps, list(range(NCORES)), trace=trace)

    yout = np.empty((B, N, C2), np.float32)
    for core in range(NCORES):
        b, half = divmod(core, 2)
        yout[b, half * TOK:(half + 1) * TOK] = res.results[core]["y"]
    yout = yout * gamma + beta
    return yout.reshape(B, HI, WI, C2), res


def kernel(**inputs):
    out, _ = run(inputs, trace=False)
    return out


# revision 11
# speedup vs baseline: 1.4425x; 1.0492x over previous
"""Cross-attention (efficient-attention variant) + 1x1 conv + LayerNorm on 8 trn2 cores.

Problem: x1,x2 [4,64,64,1024] f32. Per batch b and head h (8 heads, 128 ch each):
  value = x1[b] channel-major, kq = x2[b] channel-major
  key = softmax(kq, tokens), query = softmax(kq, head-channels)
  S = query @ key^T  [128,128];  att = S @ value  -> agg [1024, 4096]
  y = w_proj[2048,1024] @ agg + b_proj; LayerNorm(2048) * gamma + beta

Sharding: core i -> batch b=i//2, token half i%2 (2048 tokens).
Phase A (per core, all 4096 tokens of its batch): E=exp(x2 tile) once (shared by
both softmaxes); Q-hat = E * 1/rowsum per head; ST_raw[h] += E_h^T-contract-Qhat_h
accumulated over 32 token tiles in PSUM. Key-softmax normalizer: rowsum(ST_raw)
== colsum(E) because Q-hat rows sum to 1, so ST = ST_raw / rowsum(ST_raw) gives
S^T exactly.
Phase B (per core, its 2048 tokens): att_cm[h] = matmul(lhsT=ST[h], rhs=V_cm
chunk) (f32r, free=512); y = sum_h att_h^T-contract-w_projT_h (f32r, free=512);
bias-add + LayerNorm stats on free dim; gamma/beta applied on host (pure
per-channel affine after the device LN).
"""

import os
import numpy as np

import concourse.bass as bass
import concourse.tile as tile
from concourse import bacc, mybir
from concourse.bass_utils import run_bass_kernel_spmd

F32 = mybir.dt.float32
F32R = mybir.dt.float32r
BF16 = mybir.dt.bfloat16
AX = mybir.AxisListType
ALU = mybir.AluOpType
ACT_F = mybir.ActivationFunctionType

B, HI, WI, C = 4, 64, 64, 1024
N = HI * WI          # 4096 tokens per batch
HEADS = 8
CH = C // HEADS      # 128 per-head channels
C2 = 2 * C           # 2048 output channels
NCORES = 8
TOK = N // 2         # 2048 tokens per core
P = 128
NT_A = N // P        # 32 token tiles in phase A
CHUNK = 512          # phase-B token chunk (matmul free dim)
NCHUNK = TOK // CHUNK
OC = C2 // 512       # output-channel chunks of 512
EPS = 1e-5

# matmul dtype knobs (env-tunable for experiments)
MM_A = os.environ.get("K_MM_A", "bf16")    # phase-A ST matmuls (free=128)
MM_B = os.environ.get("K_MM_B", "bf16")   # phase-B matmuls (free=512)
USE_TTR = os.environ.get("K_TTR", "0") == "1"

_compiled = {}


def build():
    nc = bacc.Bacc("TRN2", target_bir_lowering=False, debug=False,
                   num_devices=NCORES)
    b_dt = {"f32r": F32R, "bf16": BF16}.get(MM_B, F32)
    xq = nc.dram_tensor("xq", [N, C], F32, kind="ExternalInput").ap()
    vcm = nc.dram_tensor("vcm", [C, TOK], b_dt, kind="ExternalInput").ap()
    wt = nc.dram_tensor("wt", [C, C2], b_dt, kind="ExternalInput").ap()
    brep = nc.dram_tensor("brep", [P, C2], F32, kind="ExternalInput").ap()
    y = nc.dram_tensor("y", [TOK, C2], F32, kind="ExternalOutput").ap()

    a_dt = BF16 if MM_A == "bf16" else F32

    with tile.TileContext(nc) as tc:
        with tc.tile_pool(name="persist", bufs=1) as persist:
            brep_sb = persist.tile([P, C2], F32)
            nc.sync.dma_start(brep_sb[:], brep[:])
            eps_sb = persist.tile([P, 1], F32, name="eps")
            nc.vector.memset(eps_sb[:], EPS)
            wt_sb = [persist.tile([P, C2], b_dt, name=f"wt{k}") for k in range(HEADS)]
            for k in range(HEADS):
                nc.sync.dma_start(wt_sb[k][:], wt[k * P:(k + 1) * P, :])
            sthat = [persist.tile([P, CH], b_dt, name=f"sthat{h}") for h in range(HEADS)]

            # ---------------- Phase A: S^T per head over all N tokens --------
            with tc.tile_pool(name="xq_p", bufs=4) as xq_p, \
                 tc.tile_pool(name="e_p", bufs=5) as e_p, \
                 tc.tile_pool(name="q_p", bufs=4) as q_p, \
                 tc.tile_pool(name="sm_a", bufs=8) as sm_a, \
                 tc.tile_pool(name="st_ps", bufs=1, space="PSUM") as st_psp:
                st_ps = [st_psp.tile([P, CH], F32, name=f"st{h}") for h in range(HEADS)]
                for nt in range(NT_A):
                    xt = xq_p.tile([P, C], F32)
                    nc.sync.dma_start(xt[:], xq[nt * P:(nt + 1) * P, :])
                    E = e_p.tile([P, C], a_dt)
                    nc.scalar.activation(E[:], xt[:], ACT_F.Exp)
                    qs = sm_a.tile([P, HEADS], F32)
                    nc.vector.reduce_sum(
                        qs[:], E.rearrange("p (h c) -> p h c", h=HEADS), axis=AX.X)
                    rq = sm_a.tile([P, HEADS], F32)
                    nc.vector.reciprocal(rq[:], qs[:])
                    Qh = q_p.tile([P, C], a_dt)
                    GH = int(os.environ.get('K_GH', '5'))  # heads on GpSimd
                    DH = HEADS - GH
                    nc.vector.tensor_tensor(
                        Qh.rearrange("p (h c) -> p h c", h=HEADS)[:, :DH],
                        E.rearrange("p (h c) -> p h c", h=HEADS)[:, :DH],
                        rq[:, :DH, None].to_broadcast([P, DH, CH]),
                        op=ALU.mult)
                    if GH:
                        nc.gpsimd.tensor_tensor(
                            Qh.rearrange("p (h c) -> p h c", h=HEADS)[:, DH:],
                            E.rearrange("p (h c) -> p h c", h=HEADS)[:, DH:],
                            rq[:, DH:, None].to_broadcast([P, GH, CH]),
                            op=ALU.mult)
                    first, last = nt == 0, nt == NT_A - 1
                    for h in range(HEADS):
                        hs = slice(h * CH, (h + 1) * CH)
                        nc.tensor.matmul(st_ps[h][:], lhsT=E[:, hs], rhs=Qh[:, hs],
                                         start=first, stop=last)
                # ST = ST_raw / rowsum(ST_raw)  (== colsum of exp-key)
                for h in range(HEADS):
                    cs = sm_a.tile([P, 1], F32, name="cs")
                    nc.vector.reduce_sum(cs[:], st_ps[h][:], axis=AX.X)
                    rc = sm_a.tile([P, 1], F32, name="rc")
                    nc.vector.reciprocal(rc[:], cs[:])
                    nc.vector.tensor_scalar_mul(sthat[h][:], st_ps[h][:], rc[:])

            # ---------------- Phase B: att, projection, LayerNorm ------------
            with tc.tile_pool(name="v_p", bufs=3) as v_p, \
                 tc.tile_pool(name="att_p", bufs=4) as att_p, \
                 tc.tile_pool(name="y_p", bufs=2) as y_p, \
                 tc.tile_pool(name="yn_p", bufs=2) as yn_p, \
                 tc.tile_pool(name="sm_b", bufs=8) as sm_b, \
                 tc.tile_pool(name="att_ps", bufs=2, space="PSUM") as att_psp, \
                 tc.tile_pool(name="ps_b", bufs=6, space="PSUM") as ps_b:
                vcm_r = vcm.rearrange("(h p) n -> p h n", p=P)
                for ck in range(NCHUNK):
                    vt = v_p.tile([P, HEADS * CHUNK], b_dt)
                    nc.sync.dma_start(
                        vt.rearrange("p (h n) -> p h n", h=HEADS),
                        vcm_r[:, :, ck * CHUNK:(ck + 1) * CHUNK])
                    att_sb = att_p.tile([P, HEADS * CHUNK], b_dt)
                    for h in range(HEADS):
                        aps = att_psp.tile([P, CHUNK], F32, tag="aps", name=f"aps{ck}_{h}")
                        nc.tensor.matmul(
                            aps[:],
                            lhsT=sthat[h][:],
                            rhs=vt[:, h * CHUNK:(h + 1) * CHUNK],
                            start=True, stop=True)
                        nc.scalar.copy(att_sb[:, h * CHUNK:(h + 1) * CHUNK], aps[:])
                    for sub in range(CHUNK // P):
                        yps = [ps_b.tile([P, 512], F32, tag="ps", name=f"yps{ck}_{sub}_{o}") for o in range(OC)]
                        for k in range(HEADS):
                            lt = att_sb[:, k * CHUNK + sub * P:k * CHUNK + (sub + 1) * P]
                            for oc in range(OC):
                                nc.tensor.matmul(
                                    yps[oc][:],
                                    lhsT=lt,
                                    rhs=wt_sb[k][:, oc * 512:(oc + 1) * 512],
                                    start=(k == 0), stop=(k == HEADS - 1))
                        ysb = y_p.tile([P, C2], F32)
                        negmu = sm_b.tile([P, 1], F32, name="negmu")
                        if USE_TTR:
                            spart = sm_b.tile([P, OC], F32, name="spart")
                            for pc in range(OC):
                                sl = slice(pc * 512, (pc + 1) * 512)
                                nc.vector.tensor_tensor_reduce(
                                    ysb[:, sl], yps[pc][:], brep_sb[:, sl],
                                    scale=1.0, scalar=0.0,
                                    op0=ALU.add, op1=ALU.add,
                                    accum_out=spart[:, pc:pc + 1])
                            nc.vector.tensor_reduce(spart[:, 0:1], spart[:], axis=AX.X,
                                                    op=ALU.add)
                            nc.vector.tensor_scalar_mul(negmu[:], spart[:, 0:1],
                                                        -1.0 / C2)
                        else:
                            for pc in range(OC):
                                sl = slice(pc * 512, (pc + 1) * 512)
                                nc.vector.tensor_tensor(
                                    ysb[:, sl], yps[pc][:], brep_sb[:, sl],
                                    op=ALU.add)
                            s_ = sm_b.tile([P, 1], F32, name="s_")
                            nc.vector.reduce_sum(s_[:], ysb[:], axis=AX.X)
                            nc.vector.tensor_scalar_mul(negmu[:], s_[:], -1.0 / C2)
                        yn = yn_p.tile([P, C2], F32)
                        ss = sm_b.tile([P, 1], F32, name="ss")
                        nc.scalar.activation(yn[:], ysb[:], ACT_F.Square,
                                             accum_out=ss[:])
                        m2 = sm_b.tile([P, 1], F32, name="m2")
                        nc.vector.tensor_tensor(m2[:], negmu[:], negmu[:], op=ALU.mult)
                        var = sm_b.tile([P, 1], F32, name="var")
                        nc.vector.tensor_scalar(var[:], ss[:], 1.0 / C2, m2[:],
                                                op0=ALU.mult, op1=ALU.subtract)
                        sig = sm_b.tile([P, 1], F32, name="sig")
                        nc.scalar.activation(sig[:], var[:], ACT_F.Sqrt,
                                             bias=eps_sb[:])
                        rsig = sm_b.tile([P, 1], F32, name="rsig")
                        nc.vector.reciprocal(rsig[:], sig[:])
                        nc.gpsimd.tensor_scalar(yn[:], ysb[:], negmu[:], rsig[:],
                                                op0=ALU.add, op1=ALU.mult)
                        row = (ck * (CHUNK // P) + sub) * P
                        nc.sync.dma_start(y[row:row + P, :], yn[:])
    nc.compile()
    return nc


def _get_nc():
    key = (MM_A, MM_B, USE_TTR)
    if key not in _compiled:
        _compiled[key] = build()
    return _compiled[key]


def run(inputs, trace=False):
    x1 = np.asarray(inputs["x1"], dtype=np.float32)
    x2 = np.asarray(inputs["x2"], dtype=np.float32)
    w_proj = np.asarray(inputs["w_proj"], dtype=np.float32)
    b_proj = np.asarray(inputs["b_proj"], dtype=np.float32)
    gamma = np.asarray(inputs["gamma"], dtype=np.float32)
    beta = np.asarray(inputs["beta"], dtype=np.float32)

    x1f = x1.reshape(B, N, C)
    x2f = x2.reshape(B, N, C)
    wt = np.ascontiguousarray(w_proj.T)                       # [C, 2C]
    brep = np.ascontiguousarray(np.broadcast_to(b_proj, (P, C2)))
    if MM_B == "bf16":
        import ml_dtypes
        wt = wt.astype(ml_dtypes.bfloat16)

    in_maps = []
    for core in range(NCORES):
        b, half = divmod(core, 2)
        vcm = np.ascontiguousarray(x1f[b].T[:, half * TOK:(half + 1) * TOK])
        if MM_B == "bf16":
            import ml_dtypes
            vcm = vcm.astype(ml_dtypes.bfloat16)
        in_maps.append({
            "xq": np.ascontiguousarray(x2f[b]),
            "vcm": vcm,
            "wt": wt,
            "brep": brep,
        })
    nc = _get_nc()
    res = run_bass_kernel_spmd(nc, in_maps, list(range(NCORES)), trace=trace)

    yout = np.empty((B, N, C2), np.float32)
    for core in range(NCORES):
        b, half = divmod(core, 2)
        yout[b, half * TOK:(half + 1) * TOK] = res.results[core]["y"]
    yout = yout * gamma + beta
    return yout.reshape(B, HI, WI, C2), res


def kernel(**inputs):
    out, _ = run(inputs, trace=False)
    return out
